# revision 27
# baseline (speedup 1.0000x reference)
"""Trainium2 Bass kernel for ContextQueryAttention (BiDAF-style trilinear
attention). Data-parallel over batch across 8 NeuronCores (4 batches/core).

Per batch (c=1024 context rows, q=128 query rows, h=256 hidden):
  S[c,q]   = ctx@cw + (qry@qw)^T + (ctx*cqw)@qry^T + bias
  S_bar    = softmax_c(S); S_bar_bar = softmax_q(S)
  A        = S @ qry
  B        = S_bar @ (S_bar_bar^T @ ctx)
  out      = concat([ctx, A, ctx*A, ctx*B], -1)

Layout/fusion strategy:
  - All heavy tensors are host-staged to bf16 in partition-major layouts so
    every DMA is a flat 4KB-per-partition copy.
  - s0 = ctx@cw rides the main S^T matmul for free: the query-side operand is
    qt_aug[h,q] = qryT[h,q]*cqw[h] + cw[h], so sum_h ctx[c,h]*qt_aug[h,q]
    = s2[c,q] + s0[c].
  - s1+bias is per-partition in the S^T [q,c] layout: it rides the Exp
    activation's bias operand (e_t) and a tensor_scalar add (st_raw).
  - Both softmax denominators come from accumulator side-outputs: zq from the
    Exp accum, zc from the transpose-copy accum. Normalizers are folded into
    the small operands (e_ss = e_sb*rc, ts = T*rq).
  - Output is assembled per 128-row tile into a [128, 4H] fp32 tile
    (chunk0=ctx on GpSimd, chunk1=A on ScalarE, chunk2/3=ctx*A/ctx*B on
    VectorE) and streamed out with per-tile DMAs (4KB rows).
"""

import numpy as np

B, C, Q, H = 32, 1024, 128, 256
N_CORES = 8
BPC = B // N_CORES  # batches per core
P = 128
HC = H // P  # h chunks of 128
CT = C // P  # c tiles of 128
CCH = 512  # S^T free-dim chunk (1 PSUM bank of fp32)
NCC = C // CCH

_NC_CACHE = {}


def _build_kernel(compile=True):
    import concourse.bacc as bacc
    import concourse.tile as tile
    from concourse import mybir
    from concourse.bass import broadcast_tensor_aps
    from concourse.masks import make_identity
    from contextlib import ExitStack

    f32 = mybir.dt.float32
    bf16 = mybir.dt.bfloat16
    AF = mybir.ActivationFunctionType
    AX = mybir.AxisListType
    ALU = mybir.AluOpType

    nc = bacc.Bacc(trn_type="TRN2", target_bir_lowering=False, debug=False)
    ctx_d = nc.dram_tensor("ctx", [BPC, P, CT * H], bf16, kind="ExternalInput").ap()
    ctxT_d = nc.dram_tensor("ctxT", [BPC, P, HC * C], bf16, kind="ExternalInput").ap()
    q_d = nc.dram_tensor("qn", [P, BPC * H], bf16, kind="ExternalInput").ap()
    qt_d = nc.dram_tensor("qt", [P, HC * BPC * Q], bf16, kind="ExternalInput").ap()
    cw_d = nc.dram_tensor("cw", [P, HC], f32, kind="ExternalInput").ap()
    cq_d = nc.dram_tensor("cq", [P, HC], f32, kind="ExternalInput").ap()
    qw_d = nc.dram_tensor("qw", [P, HC], bf16, kind="ExternalInput").ap()
    bias_d = nc.dram_tensor("bias", [P, 1], f32, kind="ExternalInput").ap()
    out_d = nc.dram_tensor("out", [BPC, C, 4 * H], f32, kind="ExternalOutput").ap()

    with tile.TileContext(nc) as tc, ExitStack() as es:
        consts = es.enter_context(tc.tile_pool(name="consts", bufs=1))
        p_ctx = es.enter_context(tc.tile_pool(name="p_ctx", bufs=2))
        p_ctxT = es.enter_context(tc.tile_pool(name="p_ctxT", bufs=2))
        p_big = es.enter_context(tc.tile_pool(name="p_big", bufs=2))
        p_med = es.enter_context(tc.tile_pool(name="p_med", bufs=2))
        p_small = es.enter_context(tc.tile_pool(name="p_small", bufs=2))
        p_bout = es.enter_context(tc.tile_pool(name="p_bout", bufs=2))
        # PSUM is 8 banks, one bank per pool buf: stp 2 + pab 2 + t_acc 2 +
        # tr 2 (s1p shares the t_acc tag's rotation).
        pp_st = es.enter_context(tc.tile_pool(name="pp_st", bufs=2, space="PSUM"))
        pp_ab = es.enter_context(tc.tile_pool(name="pp_ab", bufs=2, space="PSUM"))
        pp_t = es.enter_context(tc.tile_pool(name="pp_t", bufs=2, space="PSUM"))
        pp_tr = es.enter_context(tc.tile_pool(name="pp_tr", bufs=2, space="PSUM"))

        # critical-path consts (needed for the first stp matmuls) issue from
        # the scalar queue so they don't serialize behind the first ctx loads
        # on the sync queue
        qt_all = consts.tile([P, HC * BPC * Q], bf16)
        nc.scalar.dma_start(out=qt_all, in_=qt_d)
        cw_col = consts.tile([P, HC], f32)
        nc.scalar.dma_start(out=cw_col, in_=cw_d)
        cq_col = consts.tile([P, HC], f32)
        nc.scalar.dma_start(out=cq_col, in_=cq_d)
        qw_col = consts.tile([P, HC], bf16)
        nc.scalar.dma_start(out=qw_col, in_=qw_d)
        bias_col = consts.tile([P, 1], f32)
        nc.scalar.dma_start(out=bias_col, in_=bias_d)
        q_all = consts.tile([P, BPC * H], bf16)
        nc.scalar.dma_start(out=q_all, in_=q_d)
        identity = consts.tile([P, P], bf16)
        make_identity(nc, identity)

        # qt_aug[h,q] = qryT*cqw + cw for ALL batches at once (the +cw fold
        # carries s0 = ctx@cw through every S matmul for free)
        qt_aug = consts.tile([P, HC * BPC * Q], bf16)
        for j in range(HC):
            nc.vector.tensor_scalar(
                qt_aug[:, j * BPC * Q : (j + 1) * BPC * Q],
                qt_all[:, j * BPC * Q : (j + 1) * BPC * Q],
                cq_col[:, j : j + 1],
                cw_col[:, j : j + 1],
                ALU.mult,
                ALU.add,
            )
        # s1 columns (+ bias) for all batches: s1b[q,b] = qry[b,q,:]@qw + bias
        s1p = pp_t.tile([P, BPC], f32, tag="t_acc")
        for b in range(BPC):
            for j in range(HC):
                nc.tensor.matmul(
                    s1p[:, b : b + 1],
                    lhsT=qt_all[:, (j * BPC + b) * Q : (j * BPC + b + 1) * Q],
                    rhs=qw_col[:, j : j + 1],
                    start=(j == 0),
                    stop=(j == HC - 1),
                )
        s1b_all = consts.tile([P, BPC], f32)
        nc.vector.tensor_scalar_add(s1b_all, s1p, bias_col)

        HT = CT // 2

        def stage1(b):
            """Loads + S^T matmuls + exp/raw-S. Emitted one batch ahead so its
            instructions interleave with the previous batch's stage2 and fill
            engine bubbles."""
            ctx = p_ctx.tile([P, CT * H], bf16, tag="ctx")
            nc.sync.dma_start(out=ctx, in_=ctx_d[b])
            ctxT = p_ctxT.tile([P, HC * C], bf16, tag="ctxT")
            for j in range(HC):
                nc.sync.dma_start(
                    out=ctxT[:, j * C : (j + 1) * C],
                    in_=ctxT_d[b, :, j * C : (j + 1) * C],
                )
            s1b = s1b_all[:, b : b + 1]

            e_t = p_big.tile([P, C], bf16, tag="e_t")
            st_raw = p_big.tile([P, C], bf16, tag="st_raw")
            rsum = p_small.tile([P, NCC], f32, tag="rsum")
            for cc in range(NCC):
                sl = slice(cc * CCH, (cc + 1) * CCH)
                stp = pp_st.tile([P, CCH], f32, tag="stp")
                for j in range(HC):
                    nc.tensor.matmul(
                        stp,
                        lhsT=qt_aug[:, (j * BPC + b) * Q : (j * BPC + b + 1) * Q],
                        rhs=ctxT[:, j * C + cc * CCH : j * C + (cc + 1) * CCH],
                        start=(j == 0),
                        stop=(j == HC - 1),
                    )
                nc.scalar.activation(
                    e_t[:, sl],
                    stp,
                    AF.Exp,
                    bias=s1b,
                    scale=1.0,
                    accum_out=rsum[:, cc : cc + 1],
                )
                nc.scalar.activation(st_raw[:, sl], stp, AF.Identity, bias=s1b)
            zq = p_small.tile([P, 1], f32, tag="zq")
            nc.vector.reduce_sum(zq, rsum, axis=AX.X)
            rq = p_small.tile([P, 1], f32, tag="rq")
            nc.vector.reciprocal(rq, zq)
            return dict(ctx=ctx, e_t=e_t, st_raw=st_raw, rq=rq)

        def stage2(b, st):
            ctx, e_t, st_raw, rq = st["ctx"], st["e_t"], st["st_raw"], st["rq"]
            # whole-batch out buffer; ctx chunk as two wide strided casts
            bout = p_bout.tile([P, CT * 4 * H], f32, tag="bout")
            bout3 = bout.rearrange("p (t x) -> p t x", x=4 * H)
            ctx3 = ctx.rearrange("p (t h) -> p t h", h=H)
            nc.scalar.copy(bout3[:, 0:HT, 0:H], ctx3[:, 0:HT, :])
            nc.vector.tensor_copy(bout3[:, HT:CT, 0:H], ctx3[:, HT:CT, :])

            # transpose e_t tiles into ONE PSUM bank; zc by wide reduces;
            # e_ss = e^T * rc rides the PSUM->SBUF copy (ACT scale)
            pe_big = pp_tr.tile([P, CT * P], bf16, tag="tr")
            pe3 = pe_big.rearrange("p (t q) -> p t q", q=P)
            for t in range(CT):
                nc.tensor.transpose(
                    pe3[:, t, :], e_t[:, t * P : (t + 1) * P], identity
                )
            zc = p_small.tile([P, CT], f32, tag="zc")
            nc.vector.reduce_sum(zc[:, 0:HT], pe3[:, 0:HT, :], axis=AX.X)
            nc.vector.reduce_sum(zc[:, HT:CT], pe3[:, HT:CT, :], axis=AX.X)
            rc = p_small.tile([P, CT], f32, tag="rc")
            nc.vector.reciprocal(rc, zc)
            e_ss = p_med.tile([P, CT * P], bf16, tag="e_ss")
            e_ss3 = e_ss.rearrange("p (t q) -> p t q", q=P)
            for t in range(CT):
                nc.scalar.activation(
                    e_ss3[:, t, :],
                    pe3[:, t, :],
                    AF.Identity,
                    scale=rc[:, t : t + 1],
                )

            # T = S_bar_bar^T @ ctx; ts = T * rq
            t_acc = pp_t.tile([P, H], f32, tag="t_acc")
            for t in range(CT):
                nc.tensor.matmul(
                    t_acc,
                    lhsT=e_ss3[:, t, :],
                    rhs=ctx3[:, t, :],
                    start=(t == 0),
                    stop=(t == CT - 1),
                )
            ts = p_small.tile([P, H], bf16, tag="ts")
            nc.vector.tensor_scalar_mul(ts, t_acc, rq)

            # per c-tile: A & B matmuls, assemble [ctx|A|ctx*A|ctx*B]
            qb = q_all[:, b * H : (b + 1) * H]
            for t in range(CT):
                sl = slice(t * P, (t + 1) * P)
                pab = pp_ab.tile([P, 2 * H], f32, tag="ab")
                nc.tensor.matmul(
                    pab[:, 0:H], lhsT=st_raw[:, sl], rhs=qb, start=True, stop=True
                )
                nc.tensor.matmul(
                    pab[:, H : 2 * H], lhsT=e_t[:, sl], rhs=ts, start=True, stop=True
                )
                if t % 2 == 0:
                    nc.scalar.copy(bout3[:, t, H : 2 * H], pab[:, 0:H])
                else:
                    nc.vector.tensor_copy(bout3[:, t, H : 2 * H], pab[:, 0:H])
                    # [ctx|A] half-store: not B-gated, issued from the scalar
                    # HW-DGE queue so it can't sit behind a B-gated store on
                    # sync — fills the mid-run DMA hole and pre-drains the
                    # last batch's tail
                    nc.scalar.dma_start(
                        out=out_d[b, (t - 1) * P : (t + 1) * P, 0 : 2 * H].rearrange(
                            "(u p) x -> p u x", p=P
                        ),
                        in_=bout3[:, t - 1 : t + 1, 0 : 2 * H],
                    )
                # [ctx*A | ctx*B] in one broadcast multiply over [P, 2, H]
                bc_ctx, bc_ab = broadcast_tensor_aps(
                    ctx3[:, t : t + 1, :], pab.rearrange("p (u h) -> p u h", h=H)
                )
                nc.vector.tensor_mul(
                    bout3[:, t, 2 * H : 4 * H].rearrange("p (u h) -> p u h", h=H),
                    bc_ctx,
                    bc_ab,
                )
                if t % 2 == 1:
                    nc.sync.dma_start(
                        out=out_d[
                            b, (t - 1) * P : (t + 1) * P, 2 * H : 4 * H
                        ].rearrange("(u p) x -> p u x", p=P),
                        in_=bout3[:, t - 1 : t + 1, 2 * H : 4 * H],
                    )

        # software pipeline: stage1(b+1) is live while stage2(b) runs; the
        # scheduler fills stage2's dependency bubbles with stage1 work
        prev = None
        for b in range(BPC):
            st = stage1(b)
            if prev is not None:
                stage2(b - 1, prev)
            prev = st
        stage2(BPC - 1, prev)

    if compile:
        nc.compile()
    return nc


def _get_nc():
    if "nc" not in _NC_CACHE:
        _NC_CACHE["nc"] = _build_kernel()
    return _NC_CACHE["nc"]


def make_in_maps(context, query, c_weight, q_weight, cq_weight, bias):
    import ml_dtypes

    bf16 = ml_dtypes.bfloat16
    context = np.ascontiguousarray(np.asarray(context, dtype=np.float32))
    query = np.ascontiguousarray(np.asarray(query, dtype=np.float32))
    cw = np.asarray(c_weight, dtype=np.float32).reshape(H)
    qw = np.asarray(q_weight, dtype=np.float32).reshape(H)
    cqw = np.asarray(cq_weight, dtype=np.float32).reshape(H)
    bs = float(np.asarray(bias, dtype=np.float32).reshape(()))

    cw_col = np.ascontiguousarray(cw.reshape(HC, P).T)
    cq_col = np.ascontiguousarray(cqw.reshape(HC, P).T)
    qw_col = np.ascontiguousarray(qw.reshape(HC, P).T).astype(bf16)
    bias_col = np.full((P, 1), bs, dtype=np.float32)

    in_maps = []
    for i in range(N_CORES):
        sl = slice(i * BPC, (i + 1) * BPC)
        ctx_i = context[sl]
        qry_i = query[sl]
        # natural, partition-major: [b, p, t*h] with row c = t*P + p
        ctx_n = np.ascontiguousarray(
            ctx_i.reshape(BPC, CT, P, H).transpose(0, 2, 1, 3).reshape(BPC, P, CT * H)
        ).astype(bf16)
        # transposed, partition-major: [b, p, j*c] with col h = j*P + p
        ctxT_i = np.ascontiguousarray(
            ctx_i.transpose(0, 2, 1)
            .reshape(BPC, HC, P, C)
            .transpose(0, 2, 1, 3)
            .reshape(BPC, P, HC * C)
        ).astype(bf16)
        # qry natural on q-partitions: [p=q, b*h]
        q_n = np.ascontiguousarray(qry_i.transpose(1, 0, 2).reshape(P, BPC * H)).astype(
            bf16
        )
        # qryT on h-partitions: [p, (j b q)]
        qt_i = np.ascontiguousarray(
            qry_i.transpose(0, 2, 1)
            .reshape(BPC, HC, P, Q)
            .transpose(2, 1, 0, 3)
            .reshape(P, HC * BPC * Q)
        ).astype(bf16)
        in_maps.append(
            {
                "ctx": ctx_n,
                "ctxT": ctxT_i,
                "qn": q_n,
                "qt": qt_i,
                "cw": cw_col,
                "cq": cq_col,
                "qw": qw_col,
                "bias": bias_col,
            }
        )
    return in_maps


def kernel(context, query, c_mask, q_mask, c_weight, q_weight, cq_weight, bias):
    from concourse import bass_utils

    nc = _get_nc()
    in_maps = make_in_maps(context, query, c_weight, q_weight, cq_weight, bias)
    res = bass_utils.run_bass_kernel_spmd(nc, in_maps, core_ids=list(range(N_CORES)))
    return np.concatenate([res.results[i]["out"] for i in range(N_CORES)], axis=0)


# revision 28
# speedup vs baseline: 1.1527x; 1.1527x over previous
"""Trainium2 Bass kernel for ContextQueryAttention (BiDAF-style trilinear
attention). Data-parallel over batch across 8 NeuronCores (4 batches/core).

Per batch (c=1024 context rows, q=128 query rows, h=256 hidden):
  S[c,q]   = ctx@cw + (qry@qw)^T + (ctx*cqw)@qry^T + bias
  S_bar    = softmax_c(S); S_bar_bar = softmax_q(S)
  A        = S @ qry
  B        = S_bar @ (S_bar_bar^T @ ctx)
  out      = concat([ctx, A, ctx*A, ctx*B], -1)

Layout/fusion strategy:
  - All heavy tensors are host-staged to bf16 in partition-major layouts so
    every DMA is a flat 4KB-per-partition copy.
  - s0 = ctx@cw rides the main S^T matmul for free: the query-side operand is
    qt_aug[h,q] = qryT[h,q]*cqw[h] + cw[h], so sum_h ctx[c,h]*qt_aug[h,q]
    = s2[c,q] + s0[c].
  - s1+bias is per-partition in the S^T [q,c] layout: it rides the Exp
    activation's bias operand (e_t) and a tensor_scalar add (st_raw).
  - Both softmax denominators come from accumulator side-outputs: zq from the
    Exp accum, zc from the transpose-copy accum. Normalizers are folded into
    the small operands (e_ss = e_sb*rc, ts = T*rq).
  - Output is assembled per 128-row tile into a [128, 4H] fp32 tile
    (chunk0=ctx on GpSimd, chunk1=A on ScalarE, chunk2/3=ctx*A/ctx*B on
    VectorE) and streamed out with per-tile DMAs (4KB rows).
"""

import numpy as np

B, C, Q, H = 32, 1024, 128, 256
N_CORES = 8
BPC = B // N_CORES  # batches per core
P = 128
HC = H // P  # h chunks of 128
CT = C // P  # c tiles of 128
CCH = 512  # S^T free-dim chunk (1 PSUM bank of fp32)
NCC = C // CCH

_NC_CACHE = {}


def _build_kernel(compile=True):
    import concourse.bacc as bacc
    import concourse.tile as tile
    from concourse import mybir
    from concourse.bass import broadcast_tensor_aps
    from concourse.masks import make_identity
    from contextlib import ExitStack

    f32 = mybir.dt.float32
    bf16 = mybir.dt.bfloat16
    AF = mybir.ActivationFunctionType
    AX = mybir.AxisListType
    ALU = mybir.AluOpType

    nc = bacc.Bacc(trn_type="TRN2", target_bir_lowering=False, debug=False)
    ctx_d = nc.dram_tensor("ctx", [BPC, P, CT * H], bf16, kind="ExternalInput").ap()
    ctxT_d = nc.dram_tensor("ctxT", [BPC, P, HC * C], bf16, kind="ExternalInput").ap()
    q_d = nc.dram_tensor("qn", [P, BPC * H], bf16, kind="ExternalInput").ap()
    qt_d = nc.dram_tensor("qt", [P, HC * BPC * Q], bf16, kind="ExternalInput").ap()
    cw_d = nc.dram_tensor("cw", [P, HC], f32, kind="ExternalInput").ap()
    cq_d = nc.dram_tensor("cq", [P, HC], f32, kind="ExternalInput").ap()
    qw_d = nc.dram_tensor("qw", [P, HC], bf16, kind="ExternalInput").ap()
    bias_d = nc.dram_tensor("bias", [P, 1], f32, kind="ExternalInput").ap()
    out_d = nc.dram_tensor("out", [BPC, C, 4 * H], f32, kind="ExternalOutput").ap()

    with tile.TileContext(nc) as tc, ExitStack() as es:
        consts = es.enter_context(tc.tile_pool(name="consts", bufs=1))
        p_ctx = es.enter_context(tc.tile_pool(name="p_ctx", bufs=2))
        p_ctxT = es.enter_context(tc.tile_pool(name="p_ctxT", bufs=2))
        p_big = es.enter_context(tc.tile_pool(name="p_big", bufs=2))
        p_med = es.enter_context(tc.tile_pool(name="p_med", bufs=2))
        p_small = es.enter_context(tc.tile_pool(name="p_small", bufs=2))
        p_bout = es.enter_context(tc.tile_pool(name="p_bout", bufs=2))
        # PSUM is 8 banks, one bank per pool buf: stp 2 + pab 2 + t_acc 2 +
        # tr 2 (s1p shares the t_acc tag's rotation).
        pp_st = es.enter_context(tc.tile_pool(name="pp_st", bufs=2, space="PSUM"))
        pp_ab = es.enter_context(tc.tile_pool(name="pp_ab", bufs=2, space="PSUM"))
        pp_t = es.enter_context(tc.tile_pool(name="pp_t", bufs=2, space="PSUM"))
        pp_tr = es.enter_context(tc.tile_pool(name="pp_tr", bufs=2, space="PSUM"))

        # critical-path consts (needed for the first stp matmuls) issue from
        # the scalar queue so they don't serialize behind the first ctx loads
        # on the sync queue
        qt_all = consts.tile([P, HC * BPC * Q], bf16)
        nc.scalar.dma_start(out=qt_all, in_=qt_d)
        cw_col = consts.tile([P, HC], f32)
        nc.scalar.dma_start(out=cw_col, in_=cw_d)
        cq_col = consts.tile([P, HC], f32)
        nc.scalar.dma_start(out=cq_col, in_=cq_d)
        qw_col = consts.tile([P, HC], bf16)
        nc.scalar.dma_start(out=qw_col, in_=qw_d)
        bias_col = consts.tile([P, 1], f32)
        nc.scalar.dma_start(out=bias_col, in_=bias_d)
        q_all = consts.tile([P, BPC * H], bf16)
        nc.scalar.dma_start(out=q_all, in_=q_d)
        identity = consts.tile([P, P], bf16)
        make_identity(nc, identity)

        # qt_aug[h,q] = qryT*cqw + cw for ALL batches at once (the +cw fold
        # carries s0 = ctx@cw through every S matmul for free)
        qt_aug = consts.tile([P, HC * BPC * Q], bf16)
        for j in range(HC):
            nc.vector.tensor_scalar(
                qt_aug[:, j * BPC * Q : (j + 1) * BPC * Q],
                qt_all[:, j * BPC * Q : (j + 1) * BPC * Q],
                cq_col[:, j : j + 1],
                cw_col[:, j : j + 1],
                ALU.mult,
                ALU.add,
            )
        # s1 columns (+ bias) for all batches: s1b[q,b] = qry[b,q,:]@qw + bias
        s1p = pp_t.tile([P, BPC], f32, tag="t_acc")
        for b in range(BPC):
            for j in range(HC):
                nc.tensor.matmul(
                    s1p[:, b : b + 1],
                    lhsT=qt_all[:, (j * BPC + b) * Q : (j * BPC + b + 1) * Q],
                    rhs=qw_col[:, j : j + 1],
                    start=(j == 0),
                    stop=(j == HC - 1),
                )
        s1b_all = consts.tile([P, BPC], f32)
        nc.vector.tensor_scalar_add(s1b_all, s1p, bias_col)

        HT = CT // 2

        def stage1(b):
            """Loads + S^T matmuls + exp/raw-S. Emitted one batch ahead so its
            instructions interleave with the previous batch's stage2 and fill
            engine bubbles."""
            ctx = p_ctx.tile([P, CT * H], bf16, tag="ctx")
            nc.sync.dma_start(out=ctx, in_=ctx_d[b])
            ctxT = p_ctxT.tile([P, HC * C], bf16, tag="ctxT")
            for j in range(HC):
                nc.sync.dma_start(
                    out=ctxT[:, j * C : (j + 1) * C],
                    in_=ctxT_d[b, :, j * C : (j + 1) * C],
                )
            s1b = s1b_all[:, b : b + 1]

            e_t = p_big.tile([P, C], bf16, tag="e_t")
            st_raw = p_big.tile([P, C], bf16, tag="st_raw")
            rsum = p_small.tile([P, NCC], f32, tag="rsum")
            for cc in range(NCC):
                sl = slice(cc * CCH, (cc + 1) * CCH)
                stp = pp_st.tile([P, CCH], f32, tag="stp")
                for j in range(HC):
                    nc.tensor.matmul(
                        stp,
                        lhsT=qt_aug[:, (j * BPC + b) * Q : (j * BPC + b + 1) * Q],
                        rhs=ctxT[:, j * C + cc * CCH : j * C + (cc + 1) * CCH],
                        start=(j == 0),
                        stop=(j == HC - 1),
                    )
                nc.scalar.activation(
                    e_t[:, sl],
                    stp,
                    AF.Exp,
                    bias=s1b,
                    scale=1.0,
                    accum_out=rsum[:, cc : cc + 1],
                )
                nc.scalar.activation(st_raw[:, sl], stp, AF.Identity, bias=s1b)
            zq = p_small.tile([P, 1], f32, tag="zq")
            nc.vector.reduce_sum(zq, rsum, axis=AX.X)
            rq = p_small.tile([P, 1], f32, tag="rq")
            nc.vector.reciprocal(rq, zq)
            return dict(ctx=ctx, e_t=e_t, st_raw=st_raw, rq=rq)

        def stage2(b, st):
            ctx, e_t, st_raw, rq = st["ctx"], st["e_t"], st["st_raw"], st["rq"]
            # whole-batch out buffer; ctx chunk as two wide strided casts
            bout = p_bout.tile([P, CT * 4 * H], f32, tag="bout")
            bout3 = bout.rearrange("p (t x) -> p t x", x=4 * H)
            ctx3 = ctx.rearrange("p (t h) -> p t h", h=H)
            nc.scalar.copy(bout3[:, 0:HT, 0:H], ctx3[:, 0:HT, :])
            nc.vector.tensor_copy(bout3[:, HT:CT, 0:H], ctx3[:, HT:CT, :])

            # transpose e_t tiles into ONE PSUM bank; zc by wide reduces;
            # e_ss = e^T * rc rides the PSUM->SBUF copy (ACT scale)
            pe_big = pp_tr.tile([P, CT * P], bf16, tag="tr")
            pe3 = pe_big.rearrange("p (t q) -> p t q", q=P)
            for t in range(CT):
                nc.tensor.transpose(
                    pe3[:, t, :], e_t[:, t * P : (t + 1) * P], identity
                )
            zc = p_small.tile([P, CT], f32, tag="zc")
            nc.vector.reduce_sum(zc[:, 0:HT], pe3[:, 0:HT, :], axis=AX.X)
            nc.vector.reduce_sum(zc[:, HT:CT], pe3[:, HT:CT, :], axis=AX.X)
            rc = p_small.tile([P, CT], f32, tag="rc")
            nc.vector.reciprocal(rc, zc)
            e_ss = p_med.tile([P, CT * P], bf16, tag="e_ss")
            e_ss3 = e_ss.rearrange("p (t q) -> p t q", q=P)
            for t in range(CT):
                nc.scalar.activation(
                    e_ss3[:, t, :],
                    pe3[:, t, :],
                    AF.Identity,
                    scale=rc[:, t : t + 1],
                )

            # T = S_bar_bar^T @ ctx; ts = T * rq
            t_acc = pp_t.tile([P, H], f32, tag="t_acc")
            for t in range(CT):
                nc.tensor.matmul(
                    t_acc,
                    lhsT=e_ss3[:, t, :],
                    rhs=ctx3[:, t, :],
                    start=(t == 0),
                    stop=(t == CT - 1),
                )
            ts = p_small.tile([P, H], bf16, tag="ts")
            nc.vector.tensor_scalar_mul(ts, t_acc, rq)

            # per c-tile: A & B matmuls, assemble [ctx|A|ctx*A|ctx*B]
            qb = q_all[:, b * H : (b + 1) * H]
            for t in range(CT):
                sl = slice(t * P, (t + 1) * P)
                pab = pp_ab.tile([P, 2 * H], f32, tag="ab")
                nc.tensor.matmul(
                    pab[:, 0:H], lhsT=st_raw[:, sl], rhs=qb, start=True, stop=True
                )
                nc.tensor.matmul(
                    pab[:, H : 2 * H], lhsT=e_t[:, sl], rhs=ts, start=True, stop=True
                )
                if t % 2 == 0:
                    nc.scalar.copy(bout3[:, t, H : 2 * H], pab[:, 0:H])
                else:
                    nc.vector.tensor_copy(bout3[:, t, H : 2 * H], pab[:, 0:H])
                # [ctx*A | ctx*B] in one broadcast multiply over [P, 2, H]
                bc_ctx, bc_ab = broadcast_tensor_aps(
                    ctx3[:, t : t + 1, :], pab.rearrange("p (u h) -> p u h", h=H)
                )
                nc.vector.tensor_mul(
                    bout3[:, t, 2 * H : 4 * H].rearrange("p (u h) -> p u h", h=H),
                    bc_ctx,
                    bc_ab,
                )
                if t % 2 == 1:
                    nc.sync.dma_start(
                        out=out_d[b, (t - 1) * P : (t + 1) * P, :].rearrange(
                            "(u p) x -> p u x", p=P
                        ),
                        in_=bout3[:, t - 1 : t + 1, :],
                    )

        # software pipeline: stage1(b+1) is live while stage2(b) runs; the
        # scheduler fills stage2's dependency bubbles with stage1 work
        prev = None
        for b in range(BPC):
            st = stage1(b)
            if prev is not None:
                stage2(b - 1, prev)
            prev = st
        stage2(BPC - 1, prev)

    if compile:
        nc.compile()
    return nc


def _get_nc():
    if "nc" not in _NC_CACHE:
        _NC_CACHE["nc"] = _build_kernel()
    return _NC_CACHE["nc"]


def make_in_maps(context, query, c_weight, q_weight, cq_weight, bias):
    import ml_dtypes

    bf16 = ml_dtypes.bfloat16
    context = np.ascontiguousarray(np.asarray(context, dtype=np.float32))
    query = np.ascontiguousarray(np.asarray(query, dtype=np.float32))
    cw = np.asarray(c_weight, dtype=np.float32).reshape(H)
    qw = np.asarray(q_weight, dtype=np.float32).reshape(H)
    cqw = np.asarray(cq_weight, dtype=np.float32).reshape(H)
    bs = float(np.asarray(bias, dtype=np.float32).reshape(()))

    cw_col = np.ascontiguousarray(cw.reshape(HC, P).T)
    cq_col = np.ascontiguousarray(cqw.reshape(HC, P).T)
    qw_col = np.ascontiguousarray(qw.reshape(HC, P).T).astype(bf16)
    bias_col = np.full((P, 1), bs, dtype=np.float32)

    in_maps = []
    for i in range(N_CORES):
        sl = slice(i * BPC, (i + 1) * BPC)
        ctx_i = context[sl]
        qry_i = query[sl]
        # natural, partition-major: [b, p, t*h] with row c = t*P + p
        ctx_n = np.ascontiguousarray(
            ctx_i.reshape(BPC, CT, P, H).transpose(0, 2, 1, 3).reshape(BPC, P, CT * H)
        ).astype(bf16)
        # transposed, partition-major: [b, p, j*c] with col h = j*P + p
        ctxT_i = np.ascontiguousarray(
            ctx_i.transpose(0, 2, 1)
            .reshape(BPC, HC, P, C)
            .transpose(0, 2, 1, 3)
            .reshape(BPC, P, HC * C)
        ).astype(bf16)
        # qry natural on q-partitions: [p=q, b*h]
        q_n = np.ascontiguousarray(qry_i.transpose(1, 0, 2).reshape(P, BPC * H)).astype(
            bf16
        )
        # qryT on h-partitions: [p, (j b q)]
        qt_i = np.ascontiguousarray(
            qry_i.transpose(0, 2, 1)
            .reshape(BPC, HC, P, Q)
            .transpose(2, 1, 0, 3)
            .reshape(P, HC * BPC * Q)
        ).astype(bf16)
        in_maps.append(
            {
                "ctx": ctx_n,
                "ctxT": ctxT_i,
                "qn": q_n,
                "qt": qt_i,
                "cw": cw_col,
                "cq": cq_col,
                "qw": qw_col,
                "bias": bias_col,
            }
        )
    return in_maps


def kernel(context, query, c_mask, q_mask, c_weight, q_weight, cq_weight, bias):
    from concourse import bass_utils

    nc = _get_nc()
    in_maps = make_in_maps(context, query, c_weight, q_weight, cq_weight, bias)
    res = bass_utils.run_bass_kernel_spmd(nc, in_maps, core_ids=list(range(N_CORES)))
    return np.concatenate([res.results[i]["out"] for i in range(N_CORES)], axis=0)
